# revision 30
# baseline (speedup 1.0000x reference)
# Trainium2 Bass kernel for nn_LogitsNew (dense_mlp).
#
#   u = gelu(x @ W_proj + b_proj)                       [B, D]
#   logits = (u @ W_u)[:, None, :] + ee @ W_e           [B, N, C]
#
# Sharding: data-parallel over batch B across 8 cores (4 batches/core).
# All data moves as bf16 (host-cast; ~0.35% norm rel err, gate is 2e-2).
#
# Layouts are packed on the host PARTITION-MAJOR ([P, kslices, cols]) so
# every HBM transfer has 2-8KB contiguous runs per partition: 1KB
# descriptors cap the 16 SDMA engines at ~205 GB/s aggregate; 4-8KB runs
# reach ~430. x and the bias ride in one combined tensor (tiny-element
# transfers cost ~1-2us each on the HWDGE).
#
# A transfer's completion semaphore fires when the SLOWEST of the 16
# SDMA engines finishes, engines round-robin packets across queued
# transfers, the ~8-deep DMA sem pool makes trigger N block its queue
# until transfer N-8 completes, and the fabric ramps ~100->430 GB/s
# over its first ~6us - so phase-1's stream uses small single-k-slice
# chunks early (fast sems during the ramp) and coarser chunks late,
# with the weight transfers (wp/wu/we_h1) emitted after so their
# packets mostly trail the JIT stream:
#   sync ring:   eet k0-lo|k0-hi|k1|k2|k3|k4-5|k6-7 (phase-1 JIT),
#                wu x2 (y)
#   scalar ring: we_h0 k0|k1|k2|k3|k4-7, xb (phase-1 JIT), wp x2 (z),
#                we_h1 x2 (phase 3)
#
# Schedule (PE-stream-bound; ~35us of matmul issue):
#   - fillers from ~5us (ones memset on DVE) keep the HAM duty-cycle
#     climbing while the first chunks land (~2.5us+ PE idle re-throttles
#     the PE to half rate).
#   - phase 1: k-outer over all 8 m-tiles, h0 column half (8 PSUM
#     banks), consuming eet + we_h0 k-slices as they stream in. Banks
#     drain (plain bf16 copies, DVE/ACT alternating) as the last
#     k-iteration walks the m-tiles.
#   - phase 2: utterance path. z = x@W_proj + b (K=1 ones matmul bias),
#     k-outer so it consumes wp chunks as they arrive; u = Gelu(z) on
#     ACT; uT via 8 PE transposes into one bank; y = uT.T @ W_u (M=4).
#     y rows are DMA-gathered into a partition-0 row tile [1, 4, C]
#     (K=1 rhs must sit at partition base 0/32), then gpsimd-broadcast.
#   - phase 3: k-inner per m-tile on the h1 half. m0-5: plain ACT drain
#     + full-C in-place DVE y-add + store, staggered every ~1.7us. m6/m7
#     close in-PSUM via K=1 y-add matmuls so the kernel tail is one
#     drain + store.

import sys

if "/opt/trn_rl_repo" not in sys.path:
    sys.path.insert(0, "/opt/trn_rl_repo")

import numpy as np
import ml_dtypes

import concourse.bass as bass
import concourse.mybir as mybir
import concourse.tile as tile
from concourse import bacc
from concourse.bass_utils import run_bass_kernel_spmd
from concourse.masks import make_identity

P = 128
B, N, D, C = 32, 256, 1024, 1024
NCORES = 8
BPC = B // NCORES          # batches per core
KT = D // P                # 8 k-tiles over the contraction dim
FD = 512                   # matmul moving free dim (one PSUM bank of fp32)
NT = N // P                # 2 n-tiles per batch
MT = BPC * NT              # 8 m-tiles per core
XBW = KT * BPC + D         # combined x/bias tensor width

F32 = mybir.dt.float32
BF16 = mybir.dt.bfloat16
GELU = mybir.ActivationFunctionType.Gelu
BF = ml_dtypes.bfloat16

_CACHE = {}


def _build():
    if "nc" in _CACHE:
        return _CACHE["nc"]

    nc = bacc.Bacc("TRN2", target_bir_lowering=False, debug=False, num_devices=NCORES)

    # host-packed inputs, all partition-major (see kernel() for packing)
    eet = nc.dram_tensor("eet", [P, KT, MT * P], BF16, kind="ExternalInput").ap()
    we = nc.dram_tensor("we", [P, 2, KT, FD], BF16, kind="ExternalInput").ap()
    wp = nc.dram_tensor("wp", [P, KT, C], BF16, kind="ExternalInput").ap()
    wu = nc.dram_tensor("wu", [P, KT, C], BF16, kind="ExternalInput").ap()
    xb = nc.dram_tensor("xb", [P, XBW], BF16, kind="ExternalInput").ap()
    out = nc.dram_tensor("logits", [MT, P, C], BF16, kind="ExternalOutput").ap()

    with tile.TileContext(nc) as tc:
        with (
            tc.tile_pool(name="const", bufs=1) as cpool,
            tc.tile_pool(name="weights", bufs=1) as wpool,
            tc.tile_pool(name="obf", bufs=1) as obfpool,
            tc.tile_pool(name="mm_ps", bufs=8, space="PSUM") as mm_ps,
        ):
            eet_sb = wpool.tile([P, KT, MT * P], BF16)
            we_sb = wpool.tile([P, 2, KT, FD], BF16)
            wp_sb = wpool.tile([P, KT, C], BF16)
            wu_sb = wpool.tile([P, KT, C], BF16)
            xb_sb = cpool.tile([P, XBW], BF16)

            # ---- DMA schedule (emission order == trigger order).
            # Small first bites: the DMA fabric ramps ~100->430 GB/s
            # over its first ~6us, so phase 1's opening chunks are tiny.
            # Phase-1's JIT stream (15 transfers) goes first so the
            # sem-pool reuse (~9 sems) makes the big wp/wu/weh1 triggers
            # wait on late phase-1 completions - phase 1 gets the ramping
            # DMA bandwidth exclusively (queued transfers round-robin at
            # packet granularity per SDMA engine, so concurrent weight
            # flows would stretch every JIT chunk's completion).
            nc.sync.dma_start(eet_sb[:, 0, 0:512], eet[:, 0, 0:512])
            nc.scalar.dma_start(we_sb[:, 0, 0:1, :], we[:, 0, 0:1, :])
            nc.sync.dma_start(eet_sb[:, 0, 512:], eet[:, 0, 512:])
            nc.scalar.dma_start(we_sb[:, 0, 1:2, :], we[:, 0, 1:2, :])
            nc.sync.dma_start(eet_sb[:, 1:2, :], eet[:, 1:2, :])
            nc.scalar.dma_start(we_sb[:, 0, 2:3, :], we[:, 0, 2:3, :])
            nc.sync.dma_start(eet_sb[:, 2:3, :], eet[:, 2:3, :])
            nc.scalar.dma_start(we_sb[:, 0, 3:4, :], we[:, 0, 3:4, :])
            nc.sync.dma_start(eet_sb[:, 3:4, :], eet[:, 3:4, :])
            nc.scalar.dma_start(we_sb[:, 0, 4:8, :], we[:, 0, 4:8, :])
            nc.sync.dma_start(eet_sb[:, 4:6, :], eet[:, 4:6, :])
            nc.scalar.dma_start(xb_sb, xb)
            nc.sync.dma_start(eet_sb[:, 6:8, :], eet[:, 6:8, :])
            nc.scalar.dma_start(wp_sb[:, 0:4, :], wp[:, 0:4, :])
            nc.sync.dma_start(wu_sb[:, 0:4, :], wu[:, 0:4, :])
            nc.scalar.dma_start(wp_sb[:, 4:8, :], wp[:, 4:8, :])
            nc.sync.dma_start(wu_sb[:, 4:8, :], wu[:, 4:8, :])
            nc.scalar.dma_start(we_sb[:, 1, 0:4, :], we[:, 1, 0:4, :])
            nc.scalar.dma_start(we_sb[:, 1, 4:8, :], we[:, 1, 4:8, :])

            xt_v = xb_sb[:, 0 : KT * BPC].rearrange("p (k b) -> p k b", b=BPC)
            b_v = xb_sb[0:1, KT * BPC : XBW]

            # ---- constants (ones on DVE: earliest-ready engine) ----
            ones = cpool.tile([P, 256], BF16)
            nc.vector.memset(ones, 1.0)
            ident_f = cpool.tile([P, P], F32)
            make_identity(nc, ident_f)

            # ---- PSUM allocation (8-slot ring; order = slot order) ----
            dummy = mm_ps.tile([P, FD], F32, tag="mm", name="dummy")    # s0
            ph1 = [
                mm_ps.tile([P, FD], F32, tag="mm", name=f"p1_{m}")     # s1-s7,s0
                for m in range(MT)
            ]
            zps = [
                mm_ps.tile([P, FD], F32, tag="mm", name=f"z_{h}")      # s1,s2
                for h in range(2)
            ]
            tp_all = mm_ps.tile([P, FD], F32, tag="mm", name="tp_all")  # s3
            yps = [
                mm_ps.tile([P, FD], F32, tag="mm", name=f"y_{h}")      # s4,s5
                for h in range(2)
            ]
            p3 = [
                mm_ps.tile([P, FD], F32, tag="mm", name=f"p3_{mt}")    # s6,s7,s0-s5
                for mt in range(MT)
            ]

            def filler(n):
                # HAM keep-warm: harmless matmuls during sub-us stalls
                for _ in range(n):
                    nc.tensor.matmul(
                        dummy[:, :256], ones[:, :P], ones[:, :256],
                        start=True, stop=True,
                    )

            obf = [
                obfpool.tile([P, C], BF16, tag=f"o{m}", name=f"obf_{m}")
                for m in range(MT)
            ]

            # ---- phase 1: k-outer, all 8 m-tiles, h0 half ----
            # bridge fillers: PE activity from ~7us until the first
            # chunks land (~10us) keeps the HAM duty ramp going
            filler(16)
            for ko in range(KT):
                for m in range(MT):
                    ms = slice(m * P, (m + 1) * P)
                    nc.tensor.matmul(
                        ph1[m],
                        eet_sb[:, ko, ms],
                        we_sb[:, 0, ko, :],
                        start=(ko == 0),
                        stop=(ko == KT - 1),
                    )
                if ko == 3:
                    filler(2)  # bridge any DMA-supply gap (HAM keep-warm)
            for m in range(MT):
                if m % 2 == 0:
                    nc.vector.tensor_copy(obf[m][:, 0:FD], ph1[m])
                else:
                    nc.scalar.copy(obf[m][:, 0:FD], ph1[m])

            # ---- phase 2: utterance path ----
            # z = x @ W_proj + b; h-outer so gelu-h0 (ACT) and the first
            # transposes overlap the PE's z-h1 half; bias via K=1 ones
            # matmul.
            u32 = cpool.tile([BPC, C], F32)
            uT = cpool.tile([P, KT, BPC], BF16)
            tpv = tp_all[:, : KT * BPC].rearrange("p (a b) -> p a b", b=BPC)
            filler(1)
            for h in range(2):
                cs = slice(h * FD, (h + 1) * FD)
                for ko in range(KT):
                    nc.tensor.matmul(
                        zps[h][:BPC], xt_v[:, ko, :], wp_sb[:, ko, cs],
                        start=(ko == 0), stop=False,
                    )
                nc.tensor.matmul(
                    zps[h][:BPC], ones[:1, :BPC], b_v[:1, cs],
                    start=False, stop=True, skip_group_check=True,
                )
                nc.scalar.activation(u32[:, cs], zps[h][:BPC], GELU)

            # uT via PE transposes into one bank (transposes ko0-3 only
            # need gelu-h0, which overlapped z-h1); y = uT.T @ W_u (M=4).
            for ko in range(KT):
                nc.tensor.transpose(
                    tp_all[:, ko * BPC : (ko + 1) * BPC],
                    u32[:BPC, ko * P : (ko + 1) * P],
                    ident_f[:BPC, :BPC],
                )
                if ko == 3:
                    nc.vector.tensor_copy(uT[:, 0:4, :], tpv[:, 0:4, :])
            filler(2)  # cover the uT copy-out latency
            nc.vector.tensor_copy(uT[:, 4:8, :], tpv[:, 4:8, :])

            y_bf = cpool.tile([BPC, C], BF16)
            for ko in range(KT):
                if ko in (2, 5):
                    filler(1)  # absorb per-group issue bubbles (HAM feed)
                for h in range(2):
                    cs = slice(h * FD, (h + 1) * FD)
                    nc.tensor.matmul(
                        yps[h][:BPC], uT[:, ko, :], wu_sb[:, ko, cs],
                        start=(ko == 0), stop=(ko == KT - 1),
                    )
            for h in range(2):
                cs = slice(h * FD, (h + 1) * FD)
                nc.scalar.copy(y_bf[:, cs], yps[h][:BPC])

            # y rows gathered to partition 0: K=1 rhs for m6/m7 plus the
            # broadcast source for the DVE adds.
            y_row = cpool.tile([1, BPC, C], BF16)
            nc.scalar.dma_start(y_row, y_bf)
            ybc = cpool.tile([P, BPC, C], BF16)
            nc.gpsimd.partition_broadcast(ybc[:, 3, 0:FD], y_row[:1, 3, 0:FD])
            for b in range(3):
                nc.gpsimd.partition_broadcast(ybc[:, b, :], y_row[:1, b, :])
            # m6/m7 (batch 3): h0 halves add on DVE and store early so
            # the kernel tail is only their h1 halves; h1 closes via K=1
            for mt in (6, 7):
                nc.vector.tensor_add(
                    obf[mt][:, 0:FD], obf[mt][:, 0:FD], ybc[:, 3, 0:FD]
                )
            nc.sync.dma_start(out[6, :, 0:FD], obf[6][:, 0:FD])
            nc.scalar.dma_start(out[7, :, 0:FD], obf[7][:, 0:FD])

            # ---- phase 3: k-inner over m-tile PAIRS (halves the
            # accumulation-group-switch stalls), h1 half ----
            for pa in range(0, MT, 2):
                pair = (pa, pa + 1)
                for ko in range(KT):
                    for mt in pair:
                        ms = slice(mt * P, (mt + 1) * P)
                        nc.tensor.matmul(
                            p3[mt],
                            eet_sb[:, ko, ms],
                            we_sb[:, 1, ko, :],
                            start=(ko == 0),
                            stop=(mt < 6 and ko == KT - 1),
                            skip_group_check=True,
                        )
                if pa < 6:
                    # plain drains, then one full-C in-place y-add on DVE
                    for mt in pair:
                        nc.scalar.copy(obf[mt][:, FD:C], p3[mt])
                        nc.vector.tensor_add(
                            obf[mt], obf[mt], ybc[:, mt // NT, :]
                        )
                        if mt % 2 == 0:
                            nc.sync.dma_start(out[mt], obf[mt])
                        else:
                            nc.scalar.dma_start(out[mt], obf[mt])
                else:
                    for mt in pair:
                        nc.tensor.matmul(
                            p3[mt],
                            ones[:1, :P],
                            y_row[:1, 3, FD:C],
                            start=False, stop=True, skip_group_check=True,
                        )
                    nc.scalar.copy(obf[6][:, FD:C], p3[6])
                    nc.scalar.dma_start(out[6, :, FD:C], obf[6][:, FD:C])
                    # tail: single DVE drain + single store (a column
                    # split would mean 512B descriptors - slower)
                    nc.vector.tensor_copy(obf[7][:, FD:C], p3[7])
                    nc.sync.dma_start(out[7, :, FD:C], obf[7][:, FD:C])

    nc.compile()
    _CACHE["nc"] = nc
    return nc


def run(inputs, trace=False, **kwargs):
    nc = _build()
    x = np.asarray(inputs["encoded_utterance"], np.float32)
    ee = np.asarray(inputs["element_embeddings"], np.float32)
    w = np.asarray(inputs["weight_matrix"], np.float32)
    wp = np.asarray(inputs["W_proj"], np.float32)
    bp = np.asarray(inputs["b_proj"], np.float32)

    # shared weight packs (partition-major, bf16)
    wu_p = np.ascontiguousarray(w[:D].reshape(KT, P, C).transpose(1, 0, 2)).astype(BF)
    we_p = np.ascontiguousarray(
        w[D:].reshape(KT, P, 2, FD).transpose(1, 2, 0, 3)
    ).astype(BF)
    wp_p = np.ascontiguousarray(wp.reshape(KT, P, C).transpose(1, 0, 2)).astype(BF)

    in_maps = []
    for i in range(NCORES):
        bs = slice(i * BPC, (i + 1) * BPC)
        # eeT: [4, 256, D] -> [m=1024, D] -> [D, m] -> [P, KT, m]
        ee_c = ee[bs].reshape(BPC * N, D)
        eet_p = np.ascontiguousarray(
            ee_c.T.reshape(KT, P, MT * P).transpose(1, 0, 2)
        ).astype(BF)
        # xb: cols 0:32 = xT k-major, partition-0 cols 32: = bias
        xt_p = x[bs].T.reshape(KT, P, BPC).transpose(1, 0, 2)
        xb_p = np.zeros((P, XBW), np.float32)
        xb_p[:, 0 : KT * BPC] = xt_p.reshape(P, KT * BPC)
        xb_p[0, KT * BPC :] = bp
        in_maps.append(
            {
                "eet": eet_p,
                "we": we_p,
                "wp": wp_p,
                "wu": wu_p,
                "xb": xb_p.astype(BF),
            }
        )

    res = run_bass_kernel_spmd(
        nc, in_maps, core_ids=list(range(NCORES)), trace=trace, **kwargs
    )
    full = np.concatenate(
        [
            np.asarray(r["logits"]).astype(np.float32).reshape(BPC, N, C)
            for r in res.results
        ],
        axis=0,
    )
    return full, res


def kernel(**inputs) -> np.ndarray:
    return run(inputs, trace=False)[0]
